# revision 13
# baseline (speedup 1.0000x reference)
"""MoE FFN (top-2 routing) Trainium2 kernel.

Strategy (expert-parallel, sparse dispatch):
  - Router (tiny: T x H x E einsum + softmax + top_k) runs via jax exactly
    mirroring the reference ops, so routing decisions / lb_loss match the
    reference bitwise.
  - Host gathers each expert's assigned tokens (~T*K/E = 1024 each) into a
    padded [C, H] batch; core e runs expert e's FFN over its batch:
        y = (silu(x @ wg.T) * (x @ wu.T)) @ wd.T
    in fp16 matmuls with fp32 PSUM accumulation.
  - Host scatter-adds w_e * y back into the full [T, H] output.

This does T*K token-expert FFN applications instead of the reference's dense
T*E, i.e. 4x fewer FLOPs.

Per-core device layout (C = token capacity, multiple of 8):
  xt  [16, 128, C]  fp16   xt[k, p, c]  = x_tokens[c, k*128+p]      (x^T tiles)
  wg  [44, 128, 16, 128]   wg[i, p, k, c] = w_gate[i*128+c, k*128+p]
  wu  same layout as wg (up_proj)
  wd  [16, 128, 44, 128]   wd[h, p, i, c] = w_down[h*128+c, i*128+p]
  out [16, 128, C]  f32    out[h, p, c] = y[c, h*128+p]
All matmuls: out[m, n] = lhsT[k, m].T @ rhs[k, n] with 128x128 stationary
weight tiles and N<=512 moving slices of x^T / hidden^T kept resident in SBUF.
"""

import numpy as np

B, S, H, I, E, TOP_K = 2, 2048, 2048, 5632, 8, 2
T = B * S
P = 128
KT = H // P  # 16  k-tiles over H (layer-1 contraction)
IT = I // P  # 44  i-tiles over I
HT = H // P  # 16  h-tiles over H (output)
NMAX = 512   # moving-dim chunk (one fp32 PSUM bank)

DEV_DT = np.float16

_compiled = {}
LAST_RESULT = None  # BassKernelResults of the most recent run (for test.py)


def _install_profile_compat():
    """Make trace=True / BASS_TRACE=1 safe in images whose antenv lacks
    axon_hooks (run_bass_kernel_spmd imports it unconditionally under axon)
    and whose S3 artifact upload is unavailable. No-ops where the real
    modules work."""
    import sys
    import types

    try:
        import antenv.axon_hooks  # noqa: F401
    except ImportError:
        mod = types.ModuleType("antenv.axon_hooks")
        mod._hook = None

        def set_axon_ntff_profile_hook(h):
            mod._hook = h

        def get_axon_ntff_profile_hook():
            return mod._hook

        mod.set_axon_ntff_profile_hook = set_axon_ntff_profile_hook
        mod.get_axon_ntff_profile_hook = get_axon_ntff_profile_hook
        sys.modules["antenv.axon_hooks"] = mod
        try:
            from trn_agent_boot.trn_boot import _ntff_profile_via_ctypes

            hook = _ntff_profile_via_ctypes("/opt/axon/libaxon_pjrt.so")
            if hook is not None:
                mod._hook = hook
        except Exception:
            pass

    try:
        from concourse import bass_utils

        real_upload = bass_utils.upload_artifacts

        def _safe_upload(tmpdir):
            try:
                return real_upload(tmpdir)
            except Exception:
                return tmpdir

        bass_utils.upload_artifacts = _safe_upload
    except Exception:
        pass


def _chunks(C):
    """Split C into the fewest <=NMAX chunks, sized as equally as possible.
    Equal sizes keep every matmul's moving dim above the ~60-cycle issue
    floor (a 512/512/40 split pays the floor on the 40-wide tail)."""
    n = -(-C // NMAX)
    base, rem = divmod(C, n)
    out, c0 = [], 0
    for j in range(n):
        cn = base + (1 if j < rem else 0)
        out.append((c0, cn))
        c0 += cn
    return out


def _build(C):
    """Build + compile the per-core Bass program for token capacity C."""
    from concourse import bacc, tile, mybir
    import concourse.bass as bass

    f32 = mybir.dt.float32
    f16 = mybir.dt.float16

    nc = bacc.Bacc(None, target_bir_lowering=False)
    xt_d = nc.declare_dram_parameter("xt", [KT, P, C], f16, isOutput=False)
    wg_d = nc.declare_dram_parameter("wg", [IT, P, KT, P], f16, isOutput=False)
    wu_d = nc.declare_dram_parameter("wu", [IT, P, KT, P], f16, isOutput=False)
    wd_d = nc.declare_dram_parameter("wd", [HT, P, IT, P], f16, isOutput=False)
    out_d = nc.declare_dram_parameter("out", [HT, P, C], f32, isOutput=True)

    chunks = _chunks(C)

    with tile.TileContext(nc) as tc:
        with (
            tc.tile_pool(name="xp", bufs=1) as xp,
            tc.tile_pool(name="hp", bufs=1) as hp,
            tc.tile_pool(name="wp", bufs=2) as wp,
            tc.tile_pool(name="ap", bufs=3) as ap,
            tc.tile_pool(name="op", bufs=3) as op,
            tc.tile_pool(name="ps", bufs=2, space=bass.MemorySpace.PSUM) as ps,
        ):
            # x^T resident in SBUF: 16 tiles of [128, C] fp16, DMA'd per
            # c-chunk so the first matmuls start before the full load lands
            xts = []
            for k in range(KT):
                xt = xp.tile([P, C], f16, tag=f"x{k}", name=f"x{k}")
                xts.append(xt)
            for (c0, cn) in chunks:
                for k in range(KT):
                    nc.gpsimd.dma_start(
                        out=xts[k][:, c0:c0 + cn], in_=xt_d[k][:, c0:c0 + cn]
                    )

            # hidden^T resident in SBUF: 44 tiles of [128, C] fp16
            hids = [
                hp.tile([P, C], f16, tag=f"h{i}", name=f"h{i}")
                for i in range(IT)
            ]

            # Layer 1: gate & up projections + silu*up, i-tile stationary
            for i in range(IT):
                wgt = wp.tile([P, KT, P], f16, tag="wg")
                wut = wp.tile([P, KT, P], f16, tag="wu")
                nc.sync.dma_start(out=wgt[:], in_=wg_d[i])
                nc.sync.dma_start(out=wut[:], in_=wu_d[i])
                for (c0, cn) in chunks:
                    pg = ps.tile([P, cn], f32, tag="pg")
                    pu = ps.tile([P, cn], f32, tag="pu")
                    for k in range(KT):
                        nc.tensor.matmul(
                            pg[:], wgt[:, k, :], xts[k][:, c0:c0 + cn],
                            start=(k == 0), stop=(k == KT - 1),
                        )
                    for k in range(KT):
                        nc.tensor.matmul(
                            pu[:], wut[:, k, :], xts[k][:, c0:c0 + cn],
                            start=(k == 0), stop=(k == KT - 1),
                        )
                    sil = ap.tile([P, cn], f32, tag="sil")
                    nc.scalar.activation(
                        sil[:], pg[:], mybir.ActivationFunctionType.Silu
                    )
                    nc.vector.tensor_tensor(
                        out=hids[i][:, c0:c0 + cn], in0=sil[:], in1=pu[:],
                        op=mybir.AluOpType.mult,
                    )

            # Layer 2: down projection, h-tile stationary
            for h in range(HT):
                wdt = wp.tile([P, IT, P], f16, tag="wd")
                nc.sync.dma_start(out=wdt[:], in_=wd_d[h])
                for (c0, cn) in chunks:
                    po = ps.tile([P, cn], f32, tag="po")
                    for i in range(IT):
                        nc.tensor.matmul(
                            po[:], wdt[:, i, :], hids[i][:, c0:c0 + cn],
                            start=(i == 0), stop=(i == IT - 1),
                        )
                    ot = op.tile([P, cn], f32, tag="ot")
                    nc.vector.tensor_copy(out=ot[:], in_=po[:])
                    nc.sync.dma_start(out=out_d[h][:, c0:c0 + cn], in_=ot[:])

    nc.compile()
    return nc


def _routing(x, gate_w):
    """Mirror the reference's router ops exactly (same jax calls/backend)."""
    import jax
    import jax.numpy as jnp

    xf = jnp.asarray(x, jnp.float32).reshape(-1, H)
    gw = jnp.asarray(gate_w, jnp.float32)
    router_logits = jnp.einsum('th,eh->te', xf, gw)
    probs = jax.nn.softmax(router_logits, axis=-1)
    top_w, top_idx = jax.lax.top_k(probs, TOP_K)
    top_w = top_w / jnp.sum(top_w, axis=-1, keepdims=True)
    expert_mask = jax.nn.one_hot(top_idx, E, dtype=jnp.float32).sum(axis=1)
    lb_loss = E * jnp.sum(expert_mask.mean(axis=0) * probs.mean(axis=0))
    return np.asarray(top_w), np.asarray(top_idx), np.asarray(lb_loss)


def _pack_weights(w_gate, w_up, w_down):
    """Cast to fp16 and tile-permute each expert's weights for the device."""
    wg = np.asarray(w_gate, np.float32).astype(DEV_DT)
    wu = np.asarray(w_up, np.float32).astype(DEV_DT)
    wd = np.asarray(w_down, np.float32).astype(DEV_DT)
    packs = []
    for e in range(E):
        # [I, H] -> [IT, Pc, KT, Pp] -> [IT, Pp, KT, Pc]
        wg_e = np.ascontiguousarray(
            wg[e].reshape(IT, P, KT, P).transpose(0, 3, 2, 1))
        wu_e = np.ascontiguousarray(
            wu[e].reshape(IT, P, KT, P).transpose(0, 3, 2, 1))
        # [H, I] -> [HT, Pc, IT, Pp] -> [HT, Pp, IT, Pc]
        wd_e = np.ascontiguousarray(
            wd[e].reshape(HT, P, IT, P).transpose(0, 3, 2, 1))
        packs.append((wg_e, wu_e, wd_e))
    return packs


def kernel(x, gate_w, w_gate, w_up, w_down):
    global LAST_RESULT
    _install_profile_compat()
    from concourse.bass_utils import run_bass_kernel_spmd

    top_w, top_idx, lb_loss = _routing(x, gate_w)

    xf = np.asarray(x, np.float32).reshape(T, H)
    token_ids = [np.nonzero((top_idx == e).any(axis=1))[0] for e in range(E)]
    counts = [len(t) for t in token_ids]
    C = max(128, -(-max(counts) // 8) * 8)
    # SBUF budget: (16 x-tiles + 44 hid-tiles) * C * 2B per partition plus
    # ~45KB of weight buffers must fit in 192KB/partition -> C <= ~1216.
    assert C <= 1216, f"unexpected routing imbalance: max expert load {max(counts)}"

    if C not in _compiled:
        _compiled[C] = _build(C)
    nc = _compiled[C]

    packs = _pack_weights(w_gate, w_up, w_down)
    xf_bf = xf.astype(DEV_DT)

    in_maps = []
    for e in range(E):
        xs = np.zeros((C, H), DEV_DT)
        xs[:counts[e]] = xf_bf[token_ids[e]]
        xt = np.ascontiguousarray(xs.reshape(C, KT, P).transpose(1, 2, 0))
        wg_e, wu_e, wd_e = packs[e]
        in_maps.append({"xt": xt, "wg": wg_e, "wu": wu_e, "wd": wd_e})

    res = run_bass_kernel_spmd(nc, in_maps, list(range(E)))
    LAST_RESULT = res

    out = np.zeros((T, H), np.float32)
    for e in range(E):
        n = counts[e]
        if n == 0:
            continue
        y = res.results[e]["out"].reshape(H, C).T[:n]  # [n, H]
        ids = token_ids[e]
        w_e = np.where(top_idx[ids] == e, top_w[ids], 0.0).sum(axis=1)
        out[ids] += w_e[:, None].astype(np.float32) * y
    return out.reshape(B, S, H), lb_loss


# revision 14
# speedup vs baseline: 1.0952x; 1.0952x over previous
"""MoE FFN (top-2 routing) Trainium2 kernel.

Strategy (expert-parallel, sparse dispatch):
  - Router (tiny: T x H x E einsum + softmax + top_k) runs via jax exactly
    mirroring the reference ops, so routing decisions / lb_loss match the
    reference bitwise.
  - Host gathers each expert's assigned tokens (~T*K/E = 1024 each) into a
    padded [C, H] batch; core e runs expert e's FFN over its batch:
        y = (silu(x @ wg.T) * (x @ wu.T)) @ wd.T
    in fp16 matmuls with fp32 PSUM accumulation.
  - Host scatter-adds w_e * y back into the full [T, H] output.

This does T*K token-expert FFN applications instead of the reference's dense
T*E, i.e. 4x fewer FLOPs.

Per-core device layout (C = token capacity, multiple of 8):
  xt  [16, 128, C]  fp16   xt[k, p, c]  = x_tokens[c, k*128+p]      (x^T tiles)
  wg  [44, 128, 16, 128]   wg[i, p, k, c] = w_gate[i*128+c, k*128+p]
  wu  same layout as wg (up_proj)
  wd  [16, 128, 44, 128]   wd[h, p, i, c] = w_down[h*128+c, i*128+p]
  out [16, 128, C]  f32    out[h, p, c] = y[c, h*128+p]
All matmuls: out[m, n] = lhsT[k, m].T @ rhs[k, n] with 128x128 stationary
weight tiles and N<=512 moving slices of x^T / hidden^T kept resident in SBUF.
"""

import numpy as np

B, S, H, I, E, TOP_K = 2, 2048, 2048, 5632, 8, 2
T = B * S
P = 128
KT = H // P  # 16  k-tiles over H (layer-1 contraction)
IT = I // P  # 44  i-tiles over I
HT = H // P  # 16  h-tiles over H (output)
NMAX = 512   # moving-dim chunk (one fp32 PSUM bank)

DEV_DT = np.float16

_compiled = {}
LAST_RESULT = None  # BassKernelResults of the most recent run (for test.py)


def _install_profile_compat():
    """Make trace=True / BASS_TRACE=1 safe in images whose antenv lacks
    axon_hooks (run_bass_kernel_spmd imports it unconditionally under axon)
    and whose S3 artifact upload is unavailable. No-ops where the real
    modules work."""
    import sys
    import types

    try:
        import antenv.axon_hooks  # noqa: F401
    except ImportError:
        mod = types.ModuleType("antenv.axon_hooks")
        mod._hook = None

        def set_axon_ntff_profile_hook(h):
            mod._hook = h

        def get_axon_ntff_profile_hook():
            return mod._hook

        mod.set_axon_ntff_profile_hook = set_axon_ntff_profile_hook
        mod.get_axon_ntff_profile_hook = get_axon_ntff_profile_hook
        sys.modules["antenv.axon_hooks"] = mod
        try:
            from trn_agent_boot.trn_boot import _ntff_profile_via_ctypes

            hook = _ntff_profile_via_ctypes("/opt/axon/libaxon_pjrt.so")
            if hook is not None:
                mod._hook = hook
        except Exception:
            pass

    try:
        from concourse import bass_utils

        real_upload = bass_utils.upload_artifacts

        def _safe_upload(tmpdir):
            try:
                return real_upload(tmpdir)
            except Exception:
                return tmpdir

        bass_utils.upload_artifacts = _safe_upload
    except Exception:
        pass


def _chunks(C):
    """Greedy max-width chunks. Measured on HW: full 512-wide chunks beat
    equal-size splits (smaller moving dims pay ~20ns/MM extra overhead,
    more than the issue-floor waste of one narrow tail chunk)."""
    out, c0 = [], 0
    while c0 < C:
        cn = min(NMAX, C - c0)
        out.append((c0, cn))
        c0 += cn
    return out


def _build(C):
    """Build + compile the per-core Bass program for token capacity C."""
    from concourse import bacc, tile, mybir
    import concourse.bass as bass

    f32 = mybir.dt.float32
    f16 = mybir.dt.float16

    nc = bacc.Bacc(None, target_bir_lowering=False)
    xt_d = nc.declare_dram_parameter("xt", [KT, P, C], f16, isOutput=False)
    wg_d = nc.declare_dram_parameter("wg", [IT, P, KT, P], f16, isOutput=False)
    wu_d = nc.declare_dram_parameter("wu", [IT, P, KT, P], f16, isOutput=False)
    wd_d = nc.declare_dram_parameter("wd", [HT, P, IT, P], f16, isOutput=False)
    out_d = nc.declare_dram_parameter("out", [HT, P, C], f32, isOutput=True)

    chunks = _chunks(C)

    with tile.TileContext(nc) as tc:
        with (
            tc.tile_pool(name="xp", bufs=1) as xp,
            tc.tile_pool(name="hp", bufs=1) as hp,
            tc.tile_pool(name="wp", bufs=2) as wp,
            tc.tile_pool(name="ap", bufs=3) as ap,
            tc.tile_pool(name="op", bufs=3) as op,
            tc.tile_pool(name="ps", bufs=2, space=bass.MemorySpace.PSUM) as ps,
        ):
            # x^T resident in SBUF: 16 tiles of [128, C] fp16, DMA'd per
            # c-chunk so the first matmuls start before the full load lands
            xts = []
            for k in range(KT):
                xt = xp.tile([P, C], f16, tag=f"x{k}", name=f"x{k}")
                xts.append(xt)
            for (c0, cn) in chunks:
                for k in range(KT):
                    nc.gpsimd.dma_start(
                        out=xts[k][:, c0:c0 + cn], in_=xt_d[k][:, c0:c0 + cn]
                    )

            # hidden^T resident in SBUF: 44 tiles of [128, C] fp16
            hids = [
                hp.tile([P, C], f16, tag=f"h{i}", name=f"h{i}")
                for i in range(IT)
            ]

            # Layer 1: gate & up projections + silu*up, i-tile stationary
            for i in range(IT):
                wgt = wp.tile([P, KT, P], f16, tag="wg")
                wut = wp.tile([P, KT, P], f16, tag="wu")
                nc.sync.dma_start(out=wgt[:], in_=wg_d[i])
                nc.sync.dma_start(out=wut[:], in_=wu_d[i])
                for (c0, cn) in chunks:
                    pg = ps.tile([P, cn], f32, tag="pg")
                    pu = ps.tile([P, cn], f32, tag="pu")
                    for k in range(KT):
                        nc.tensor.matmul(
                            pg[:], wgt[:, k, :], xts[k][:, c0:c0 + cn],
                            start=(k == 0), stop=(k == KT - 1),
                        )
                    for k in range(KT):
                        nc.tensor.matmul(
                            pu[:], wut[:, k, :], xts[k][:, c0:c0 + cn],
                            start=(k == 0), stop=(k == KT - 1),
                        )
                    sil = ap.tile([P, cn], f32, tag="sil")
                    nc.scalar.activation(
                        sil[:], pg[:], mybir.ActivationFunctionType.Silu
                    )
                    nc.vector.tensor_tensor(
                        out=hids[i][:, c0:c0 + cn], in0=sil[:], in1=pu[:],
                        op=mybir.AluOpType.mult,
                    )

            # Layer 2: down projection, h-tile stationary
            for h in range(HT):
                wdt = wp.tile([P, IT, P], f16, tag="wd")
                nc.sync.dma_start(out=wdt[:], in_=wd_d[h])
                for (c0, cn) in chunks:
                    po = ps.tile([P, cn], f32, tag="po")
                    for i in range(IT):
                        nc.tensor.matmul(
                            po[:], wdt[:, i, :], hids[i][:, c0:c0 + cn],
                            start=(i == 0), stop=(i == IT - 1),
                        )
                    ot = op.tile([P, cn], f32, tag="ot")
                    nc.vector.tensor_copy(out=ot[:], in_=po[:])
                    nc.sync.dma_start(out=out_d[h][:, c0:c0 + cn], in_=ot[:])

    nc.compile()
    return nc


def _routing(x, gate_w):
    """Mirror the reference's router ops exactly (same jax calls/backend)."""
    import jax
    import jax.numpy as jnp

    xf = jnp.asarray(x, jnp.float32).reshape(-1, H)
    gw = jnp.asarray(gate_w, jnp.float32)
    router_logits = jnp.einsum('th,eh->te', xf, gw)
    probs = jax.nn.softmax(router_logits, axis=-1)
    top_w, top_idx = jax.lax.top_k(probs, TOP_K)
    top_w = top_w / jnp.sum(top_w, axis=-1, keepdims=True)
    expert_mask = jax.nn.one_hot(top_idx, E, dtype=jnp.float32).sum(axis=1)
    lb_loss = E * jnp.sum(expert_mask.mean(axis=0) * probs.mean(axis=0))
    return np.asarray(top_w), np.asarray(top_idx), np.asarray(lb_loss)


def _pack_weights(w_gate, w_up, w_down):
    """Cast to fp16 and tile-permute each expert's weights for the device."""
    wg = np.asarray(w_gate, np.float32).astype(DEV_DT)
    wu = np.asarray(w_up, np.float32).astype(DEV_DT)
    wd = np.asarray(w_down, np.float32).astype(DEV_DT)
    packs = []
    for e in range(E):
        # [I, H] -> [IT, Pc, KT, Pp] -> [IT, Pp, KT, Pc]
        wg_e = np.ascontiguousarray(
            wg[e].reshape(IT, P, KT, P).transpose(0, 3, 2, 1))
        wu_e = np.ascontiguousarray(
            wu[e].reshape(IT, P, KT, P).transpose(0, 3, 2, 1))
        # [H, I] -> [HT, Pc, IT, Pp] -> [HT, Pp, IT, Pc]
        wd_e = np.ascontiguousarray(
            wd[e].reshape(HT, P, IT, P).transpose(0, 3, 2, 1))
        packs.append((wg_e, wu_e, wd_e))
    return packs


def kernel(x, gate_w, w_gate, w_up, w_down):
    global LAST_RESULT
    _install_profile_compat()
    from concourse.bass_utils import run_bass_kernel_spmd

    top_w, top_idx, lb_loss = _routing(x, gate_w)

    xf = np.asarray(x, np.float32).reshape(T, H)
    token_ids = [np.nonzero((top_idx == e).any(axis=1))[0] for e in range(E)]
    counts = [len(t) for t in token_ids]
    C = max(128, -(-max(counts) // 8) * 8)
    # SBUF budget: (16 x-tiles + 44 hid-tiles) * C * 2B per partition plus
    # ~45KB of weight buffers must fit in 192KB/partition -> C <= ~1216.
    assert C <= 1216, f"unexpected routing imbalance: max expert load {max(counts)}"

    if C not in _compiled:
        _compiled[C] = _build(C)
    nc = _compiled[C]

    packs = _pack_weights(w_gate, w_up, w_down)
    xf_bf = xf.astype(DEV_DT)

    in_maps = []
    for e in range(E):
        xs = np.zeros((C, H), DEV_DT)
        xs[:counts[e]] = xf_bf[token_ids[e]]
        xt = np.ascontiguousarray(xs.reshape(C, KT, P).transpose(1, 2, 0))
        wg_e, wu_e, wd_e = packs[e]
        in_maps.append({"xt": xt, "wg": wg_e, "wu": wu_e, "wd": wd_e})

    res = run_bass_kernel_spmd(nc, in_maps, list(range(E)))
    LAST_RESULT = res

    out = np.zeros((T, H), np.float32)
    for e in range(E):
        n = counts[e]
        if n == 0:
            continue
        y = res.results[e]["out"].reshape(H, C).T[:n]  # [n, H]
        ids = token_ids[e]
        w_e = np.where(top_idx[ids] == e, top_w[ids], 0.0).sum(axis=1)
        out[ids] += w_e[:, None].astype(np.float32) * y
    return out.reshape(B, S, H), lb_loss


# revision 17
# speedup vs baseline: 1.0961x; 1.0008x over previous
"""MoE FFN (top-2 routing) Trainium2 kernel.

Strategy (expert-parallel, sparse dispatch):
  - Router (tiny: T x H x E einsum + softmax + top_k) runs via jax exactly
    mirroring the reference ops, so routing decisions / lb_loss match the
    reference bitwise.
  - Host gathers each expert's assigned tokens (~T*K/E = 1024 each) into a
    padded [C, H] batch; core e runs expert e's FFN over its batch:
        y = (silu(x @ wg.T) * (x @ wu.T)) @ wd.T
    in fp16 matmuls with fp32 PSUM accumulation.
  - Host scatter-adds w_e * y back into the full [T, H] output.

This does T*K token-expert FFN applications instead of the reference's dense
T*E, i.e. 4x fewer FLOPs.

Per-core device layout (C = token capacity, multiple of 8):
  xt  [16, 128, C]  fp16   xt[k, p, c]  = x_tokens[c, k*128+p]      (x^T tiles)
  wg  [44, 128, 16, 128]   wg[i, p, k, c] = w_gate[i*128+c, k*128+p]
  wu  same layout as wg (up_proj)
  wd  [16, 128, 44, 128]   wd[h, p, i, c] = w_down[h*128+c, i*128+p]
  out [16, 128, C]  f32    out[h, p, c] = y[c, h*128+p]
All matmuls: out[m, n] = lhsT[k, m].T @ rhs[k, n] with 128x128 stationary
weight tiles and N<=512 moving slices of x^T / hidden^T kept resident in SBUF.
"""

import numpy as np

B, S, H, I, E, TOP_K = 2, 2048, 2048, 5632, 8, 2
T = B * S
P = 128
KT = H // P  # 16  k-tiles over H (layer-1 contraction)
IT = I // P  # 44  i-tiles over I
HT = H // P  # 16  h-tiles over H (output)
NMAX = 512   # moving-dim chunk (one fp32 PSUM bank)

DEV_DT = np.float16

_compiled = {}
LAST_RESULT = None  # BassKernelResults of the most recent run (for test.py)


def _install_profile_compat():
    """Make trace=True / BASS_TRACE=1 safe in images whose antenv lacks
    axon_hooks (run_bass_kernel_spmd imports it unconditionally under axon)
    and whose S3 artifact upload is unavailable. No-ops where the real
    modules work."""
    import sys
    import types

    try:
        import antenv.axon_hooks  # noqa: F401
    except ImportError:
        mod = types.ModuleType("antenv.axon_hooks")
        mod._hook = None

        def set_axon_ntff_profile_hook(h):
            mod._hook = h

        def get_axon_ntff_profile_hook():
            return mod._hook

        mod.set_axon_ntff_profile_hook = set_axon_ntff_profile_hook
        mod.get_axon_ntff_profile_hook = get_axon_ntff_profile_hook
        sys.modules["antenv.axon_hooks"] = mod
        try:
            from trn_agent_boot.trn_boot import _ntff_profile_via_ctypes

            hook = _ntff_profile_via_ctypes("/opt/axon/libaxon_pjrt.so")
            if hook is not None:
                mod._hook = hook
        except Exception:
            pass

    try:
        from concourse import bass_utils

        real_upload = bass_utils.upload_artifacts

        def _safe_upload(tmpdir):
            try:
                return real_upload(tmpdir)
            except Exception:
                return tmpdir

        bass_utils.upload_artifacts = _safe_upload
    except Exception:
        pass


def _chunks(C):
    """Greedy max-width chunks. Measured on HW: full 512-wide chunks beat
    equal-size splits (smaller moving dims pay ~20ns/MM extra overhead,
    more than the issue-floor waste of one narrow tail chunk)."""
    out, c0 = [], 0
    while c0 < C:
        cn = min(NMAX, C - c0)
        out.append((c0, cn))
        c0 += cn
    return out


def _build(C):
    """Build + compile the per-core Bass program for token capacity C."""
    from concourse import bacc, tile, mybir
    import concourse.bass as bass

    f32 = mybir.dt.float32
    f16 = mybir.dt.float16

    nc = bacc.Bacc(None, target_bir_lowering=False)
    xt_d = nc.declare_dram_parameter("xt", [KT, P, C], f16, isOutput=False)
    wg_d = nc.declare_dram_parameter("wg", [IT, P, KT, P], f16, isOutput=False)
    wu_d = nc.declare_dram_parameter("wu", [IT, P, KT, P], f16, isOutput=False)
    wd_d = nc.declare_dram_parameter("wd", [HT, P, IT, P], f16, isOutput=False)
    out_d = nc.declare_dram_parameter("out", [HT, P, C], f32, isOutput=True)

    chunks = _chunks(C)

    with tile.TileContext(nc) as tc:
        with (
            tc.tile_pool(name="xp", bufs=1) as xp,
            tc.tile_pool(name="hp", bufs=1) as hp,
            tc.tile_pool(name="wp", bufs=2) as wp,
            tc.tile_pool(name="ap", bufs=3) as ap,
            tc.tile_pool(name="op", bufs=3) as op,
            tc.tile_pool(name="ps", bufs=2, space=bass.MemorySpace.PSUM) as ps,
        ):
            # x^T resident in SBUF as one tile per (k, c-chunk) so each
            # matmul depends on exactly one DMA (a single [128, C] tile would
            # make the first matmul wait for the whole x load)
            xts = [
                [
                    xp.tile([P, cn], f16, tag=f"x{k}_{ci}", name=f"x{k}_{ci}")
                    for ci, (c0, cn) in enumerate(chunks)
                ]
                for k in range(KT)
            ]
            for ci, (c0, cn) in enumerate(chunks):
                for k in range(KT):
                    nc.gpsimd.dma_start(
                        out=xts[k][ci][:], in_=xt_d[k][:, c0:c0 + cn]
                    )

            # hidden^T resident in SBUF, also tiled per (i, c-chunk)
            hids = [
                [
                    hp.tile([P, cn], f16, tag=f"h{i}_{ci}", name=f"h{i}_{ci}")
                    for ci, (c0, cn) in enumerate(chunks)
                ]
                for i in range(IT)
            ]

            # Layer 1: gate & up projections + silu*up, i-tile stationary
            for i in range(IT):
                wgt = wp.tile([P, KT, P], f16, tag="wg")
                wut = wp.tile([P, KT, P], f16, tag="wu")
                nc.sync.dma_start(out=wgt[:], in_=wg_d[i])
                nc.sync.dma_start(out=wut[:], in_=wu_d[i])
                for ci, (c0, cn) in enumerate(chunks):
                    pg = ps.tile([P, cn], f32, tag="pg")
                    pu = ps.tile([P, cn], f32, tag="pu")
                    for k in range(KT):
                        nc.tensor.matmul(
                            pg[:], wgt[:, k, :], xts[k][ci][:],
                            start=(k == 0), stop=(k == KT - 1),
                        )
                    for k in range(KT):
                        nc.tensor.matmul(
                            pu[:], wut[:, k, :], xts[k][ci][:],
                            start=(k == 0), stop=(k == KT - 1),
                        )
                    sil = ap.tile([P, cn], f32, tag="sil")
                    nc.scalar.activation(
                        sil[:], pg[:], mybir.ActivationFunctionType.Silu
                    )
                    nc.vector.tensor_tensor(
                        out=hids[i][ci][:], in0=sil[:], in1=pu[:],
                        op=mybir.AluOpType.mult,
                    )

            # Layer 2: down projection, h-tile stationary
            for h in range(HT):
                wdt = wp.tile([P, IT, P], f16, tag="wd")
                nc.sync.dma_start(out=wdt[:], in_=wd_d[h])
                for ci, (c0, cn) in enumerate(chunks):
                    po = ps.tile([P, cn], f32, tag="po")
                    for i in range(IT):
                        nc.tensor.matmul(
                            po[:], wdt[:, i, :], hids[i][ci][:],
                            start=(i == 0), stop=(i == IT - 1),
                        )
                    ot = op.tile([P, cn], f32, tag="ot")
                    nc.vector.tensor_copy(out=ot[:], in_=po[:])
                    nc.sync.dma_start(out=out_d[h][:, c0:c0 + cn], in_=ot[:])

    nc.compile()
    return nc


def _routing(x, gate_w):
    """Mirror the reference's router ops exactly (same jax calls/backend)."""
    import jax
    import jax.numpy as jnp

    xf = jnp.asarray(x, jnp.float32).reshape(-1, H)
    gw = jnp.asarray(gate_w, jnp.float32)
    router_logits = jnp.einsum('th,eh->te', xf, gw)
    probs = jax.nn.softmax(router_logits, axis=-1)
    top_w, top_idx = jax.lax.top_k(probs, TOP_K)
    top_w = top_w / jnp.sum(top_w, axis=-1, keepdims=True)
    expert_mask = jax.nn.one_hot(top_idx, E, dtype=jnp.float32).sum(axis=1)
    lb_loss = E * jnp.sum(expert_mask.mean(axis=0) * probs.mean(axis=0))
    return np.asarray(top_w), np.asarray(top_idx), np.asarray(lb_loss)


def _pack_weights(w_gate, w_up, w_down):
    """Cast to fp16 and tile-permute each expert's weights for the device."""
    wg = np.asarray(w_gate, np.float32).astype(DEV_DT)
    wu = np.asarray(w_up, np.float32).astype(DEV_DT)
    wd = np.asarray(w_down, np.float32).astype(DEV_DT)
    packs = []
    for e in range(E):
        # [I, H] -> [IT, Pc, KT, Pp] -> [IT, Pp, KT, Pc]
        wg_e = np.ascontiguousarray(
            wg[e].reshape(IT, P, KT, P).transpose(0, 3, 2, 1))
        wu_e = np.ascontiguousarray(
            wu[e].reshape(IT, P, KT, P).transpose(0, 3, 2, 1))
        # [H, I] -> [HT, Pc, IT, Pp] -> [HT, Pp, IT, Pc]
        wd_e = np.ascontiguousarray(
            wd[e].reshape(HT, P, IT, P).transpose(0, 3, 2, 1))
        packs.append((wg_e, wu_e, wd_e))
    return packs


def kernel(x, gate_w, w_gate, w_up, w_down):
    global LAST_RESULT
    _install_profile_compat()
    from concourse.bass_utils import run_bass_kernel_spmd

    top_w, top_idx, lb_loss = _routing(x, gate_w)

    xf = np.asarray(x, np.float32).reshape(T, H)
    token_ids = [np.nonzero((top_idx == e).any(axis=1))[0] for e in range(E)]
    counts = [len(t) for t in token_ids]
    C = max(128, -(-max(counts) // 8) * 8)
    # SBUF budget: (16 x-tiles + 44 hid-tiles) * C * 2B per partition plus
    # ~45KB of weight buffers must fit in 192KB/partition -> C <= ~1216.
    assert C <= 1216, f"unexpected routing imbalance: max expert load {max(counts)}"

    if C not in _compiled:
        _compiled[C] = _build(C)
    nc = _compiled[C]

    packs = _pack_weights(w_gate, w_up, w_down)
    xf_bf = xf.astype(DEV_DT)

    in_maps = []
    for e in range(E):
        xs = np.zeros((C, H), DEV_DT)
        xs[:counts[e]] = xf_bf[token_ids[e]]
        xt = np.ascontiguousarray(xs.reshape(C, KT, P).transpose(1, 2, 0))
        wg_e, wu_e, wd_e = packs[e]
        in_maps.append({"xt": xt, "wg": wg_e, "wu": wu_e, "wd": wd_e})

    res = run_bass_kernel_spmd(nc, in_maps, list(range(E)))
    LAST_RESULT = res

    out = np.zeros((T, H), np.float32)
    for e in range(E):
        n = counts[e]
        if n == 0:
            continue
        y = res.results[e]["out"].reshape(H, C).T[:n]  # [n, H]
        ids = token_ids[e]
        w_e = np.where(top_idx[ids] == e, top_w[ids], 0.0).sum(axis=1)
        out[ids] += w_e[:, None].astype(np.float32) * y
    return out.reshape(B, S, H), lb_loss


# revision 18
# speedup vs baseline: 1.0969x; 1.0007x over previous
"""MoE FFN (top-2 routing) Trainium2 kernel.

Strategy (expert-parallel, sparse dispatch):
  - Router (tiny: T x H x E einsum + softmax + top_k) runs via jax exactly
    mirroring the reference ops, so routing decisions / lb_loss match the
    reference bitwise.
  - Host gathers each expert's assigned tokens (~T*K/E = 1024 each) into a
    padded [C, H] batch; core e runs expert e's FFN over its batch:
        y = (silu(x @ wg.T) * (x @ wu.T)) @ wd.T
    in fp16 matmuls with fp32 PSUM accumulation.
  - Host scatter-adds w_e * y back into the full [T, H] output.

This does T*K token-expert FFN applications instead of the reference's dense
T*E, i.e. 4x fewer FLOPs.

Per-core device layout (C = token capacity, multiple of 8):
  xt  [16, 128, C]  fp16   xt[k, p, c]  = x_tokens[c, k*128+p]      (x^T tiles)
  wg  [44, 128, 16, 128]   wg[i, p, k, c] = w_gate[i*128+c, k*128+p]
  wu  same layout as wg (up_proj)
  wd  [16, 128, 44, 128]   wd[h, p, i, c] = w_down[h*128+c, i*128+p]
  out [16, 128, C]  f32    out[h, p, c] = y[c, h*128+p]
All matmuls: out[m, n] = lhsT[k, m].T @ rhs[k, n] with 128x128 stationary
weight tiles and N<=512 moving slices of x^T / hidden^T kept resident in SBUF.
"""

import numpy as np

B, S, H, I, E, TOP_K = 2, 2048, 2048, 5632, 8, 2
T = B * S
P = 128
KT = H // P  # 16  k-tiles over H (layer-1 contraction)
IT = I // P  # 44  i-tiles over I
HT = H // P  # 16  h-tiles over H (output)
NMAX = 512   # moving-dim chunk (one fp32 PSUM bank)

DEV_DT = np.float16

_compiled = {}
LAST_RESULT = None  # BassKernelResults of the most recent run (for test.py)


def _install_profile_compat():
    """Make trace=True / BASS_TRACE=1 safe in images whose antenv lacks
    axon_hooks (run_bass_kernel_spmd imports it unconditionally under axon)
    and whose S3 artifact upload is unavailable. No-ops where the real
    modules work."""
    import sys
    import types

    try:
        import antenv.axon_hooks  # noqa: F401
    except ImportError:
        mod = types.ModuleType("antenv.axon_hooks")
        mod._hook = None

        def set_axon_ntff_profile_hook(h):
            mod._hook = h

        def get_axon_ntff_profile_hook():
            return mod._hook

        mod.set_axon_ntff_profile_hook = set_axon_ntff_profile_hook
        mod.get_axon_ntff_profile_hook = get_axon_ntff_profile_hook
        sys.modules["antenv.axon_hooks"] = mod
        try:
            from trn_agent_boot.trn_boot import _ntff_profile_via_ctypes

            hook = _ntff_profile_via_ctypes("/opt/axon/libaxon_pjrt.so")
            if hook is not None:
                mod._hook = hook
        except Exception:
            pass

    try:
        from concourse import bass_utils

        real_upload = bass_utils.upload_artifacts

        def _safe_upload(tmpdir):
            try:
                return real_upload(tmpdir)
            except Exception:
                return tmpdir

        bass_utils.upload_artifacts = _safe_upload
    except Exception:
        pass


def _chunks(C):
    """Greedy max-width chunks. Measured on HW: full 512-wide chunks beat
    equal-size splits (smaller moving dims pay ~20ns/MM extra overhead,
    more than the issue-floor waste of one narrow tail chunk)."""
    out, c0 = [], 0
    while c0 < C:
        cn = min(NMAX, C - c0)
        out.append((c0, cn))
        c0 += cn
    return out


def _build(C):
    """Build + compile the per-core Bass program for token capacity C."""
    from concourse import bacc, tile, mybir
    import concourse.bass as bass

    f32 = mybir.dt.float32
    f16 = mybir.dt.float16

    nc = bacc.Bacc(None, target_bir_lowering=False)
    xt_d = nc.declare_dram_parameter("xt", [KT, P, C], f16, isOutput=False)
    wg_d = nc.declare_dram_parameter("wg", [IT, P, KT, P], f16, isOutput=False)
    wu_d = nc.declare_dram_parameter("wu", [IT, P, KT, P], f16, isOutput=False)
    wd_d = nc.declare_dram_parameter("wd", [HT, P, IT, P], f16, isOutput=False)
    out_d = nc.declare_dram_parameter("out", [HT, P, C], f32, isOutput=True)

    chunks = _chunks(C)

    with tile.TileContext(nc) as tc:
        with (
            tc.tile_pool(name="xp", bufs=1) as xp,
            tc.tile_pool(name="hp", bufs=1) as hp,
            tc.tile_pool(name="wp", bufs=2) as wp,
            tc.tile_pool(name="ap", bufs=3) as ap,
            tc.tile_pool(name="op", bufs=3) as op,
            tc.tile_pool(name="ps", bufs=2, space=bass.MemorySpace.PSUM) as ps,
        ):
            # x^T resident in SBUF as one tile per (k, c-chunk) so each
            # matmul depends on exactly one DMA (a single [128, C] tile would
            # make the first matmul wait for the whole x load)
            xts = [
                [
                    xp.tile([P, cn], f16, tag=f"x{k}_{ci}", name=f"x{k}_{ci}")
                    for ci, (c0, cn) in enumerate(chunks)
                ]
                for k in range(KT)
            ]
            for ci, (c0, cn) in enumerate(chunks):
                for k in range(KT):
                    nc.gpsimd.dma_start(
                        out=xts[k][ci][:], in_=xt_d[k][:, c0:c0 + cn]
                    )

            # hidden^T resident in SBUF, also tiled per (i, c-chunk)
            hids = [
                [
                    hp.tile([P, cn], f16, tag=f"h{i}_{ci}", name=f"h{i}_{ci}")
                    for ci, (c0, cn) in enumerate(chunks)
                ]
                for i in range(IT)
            ]

            # Layer 1: gate & up projections + silu*up, i-tile stationary
            for i in range(IT):
                wgt = wp.tile([P, KT, P], f16, tag="wg")
                wut = wp.tile([P, KT, P], f16, tag="wu")
                nc.sync.dma_start(out=wgt[:], in_=wg_d[i])
                nc.sync.dma_start(out=wut[:], in_=wu_d[i])
                for ci, (c0, cn) in enumerate(chunks):
                    pg = ps.tile([P, cn], f32, tag="pg", bufs=3)
                    pu = ps.tile([P, cn], f32, tag="pu", bufs=3)
                    for k in range(KT):
                        nc.tensor.matmul(
                            pg[:], wgt[:, k, :], xts[k][ci][:],
                            start=(k == 0), stop=(k == KT - 1),
                        )
                    for k in range(KT):
                        nc.tensor.matmul(
                            pu[:], wut[:, k, :], xts[k][ci][:],
                            start=(k == 0), stop=(k == KT - 1),
                        )
                    sil = ap.tile([P, cn], f32, tag="sil")
                    nc.scalar.activation(
                        sil[:], pg[:], mybir.ActivationFunctionType.Silu
                    )
                    nc.vector.tensor_tensor(
                        out=hids[i][ci][:], in0=sil[:], in1=pu[:],
                        op=mybir.AluOpType.mult,
                    )

            # Layer 2: down projection, h-tile stationary
            for h in range(HT):
                wdt = wp.tile([P, IT, P], f16, tag="wd")
                nc.sync.dma_start(out=wdt[:], in_=wd_d[h])
                for ci, (c0, cn) in enumerate(chunks):
                    po = ps.tile([P, cn], f32, tag="po")
                    for i in range(IT):
                        nc.tensor.matmul(
                            po[:], wdt[:, i, :], hids[i][ci][:],
                            start=(i == 0), stop=(i == IT - 1),
                        )
                    ot = op.tile([P, cn], f32, tag="ot")
                    nc.vector.tensor_copy(out=ot[:], in_=po[:])
                    nc.sync.dma_start(out=out_d[h][:, c0:c0 + cn], in_=ot[:])

    nc.compile()
    return nc


def _routing(x, gate_w):
    """Mirror the reference's router ops exactly (same jax calls/backend)."""
    import jax
    import jax.numpy as jnp

    xf = jnp.asarray(x, jnp.float32).reshape(-1, H)
    gw = jnp.asarray(gate_w, jnp.float32)
    router_logits = jnp.einsum('th,eh->te', xf, gw)
    probs = jax.nn.softmax(router_logits, axis=-1)
    top_w, top_idx = jax.lax.top_k(probs, TOP_K)
    top_w = top_w / jnp.sum(top_w, axis=-1, keepdims=True)
    expert_mask = jax.nn.one_hot(top_idx, E, dtype=jnp.float32).sum(axis=1)
    lb_loss = E * jnp.sum(expert_mask.mean(axis=0) * probs.mean(axis=0))
    return np.asarray(top_w), np.asarray(top_idx), np.asarray(lb_loss)


def _pack_weights(w_gate, w_up, w_down):
    """Cast to fp16 and tile-permute each expert's weights for the device."""
    wg = np.asarray(w_gate, np.float32).astype(DEV_DT)
    wu = np.asarray(w_up, np.float32).astype(DEV_DT)
    wd = np.asarray(w_down, np.float32).astype(DEV_DT)
    packs = []
    for e in range(E):
        # [I, H] -> [IT, Pc, KT, Pp] -> [IT, Pp, KT, Pc]
        wg_e = np.ascontiguousarray(
            wg[e].reshape(IT, P, KT, P).transpose(0, 3, 2, 1))
        wu_e = np.ascontiguousarray(
            wu[e].reshape(IT, P, KT, P).transpose(0, 3, 2, 1))
        # [H, I] -> [HT, Pc, IT, Pp] -> [HT, Pp, IT, Pc]
        wd_e = np.ascontiguousarray(
            wd[e].reshape(HT, P, IT, P).transpose(0, 3, 2, 1))
        packs.append((wg_e, wu_e, wd_e))
    return packs


def kernel(x, gate_w, w_gate, w_up, w_down):
    global LAST_RESULT
    _install_profile_compat()
    from concourse.bass_utils import run_bass_kernel_spmd

    top_w, top_idx, lb_loss = _routing(x, gate_w)

    xf = np.asarray(x, np.float32).reshape(T, H)
    token_ids = [np.nonzero((top_idx == e).any(axis=1))[0] for e in range(E)]
    counts = [len(t) for t in token_ids]
    C = max(128, -(-max(counts) // 8) * 8)
    # SBUF budget: (16 x-tiles + 44 hid-tiles) * C * 2B per partition plus
    # ~45KB of weight buffers must fit in 192KB/partition -> C <= ~1216.
    assert C <= 1216, f"unexpected routing imbalance: max expert load {max(counts)}"

    if C not in _compiled:
        _compiled[C] = _build(C)
    nc = _compiled[C]

    packs = _pack_weights(w_gate, w_up, w_down)
    xf_bf = xf.astype(DEV_DT)

    in_maps = []
    for e in range(E):
        xs = np.zeros((C, H), DEV_DT)
        xs[:counts[e]] = xf_bf[token_ids[e]]
        xt = np.ascontiguousarray(xs.reshape(C, KT, P).transpose(1, 2, 0))
        wg_e, wu_e, wd_e = packs[e]
        in_maps.append({"xt": xt, "wg": wg_e, "wu": wu_e, "wd": wd_e})

    res = run_bass_kernel_spmd(nc, in_maps, list(range(E)))
    LAST_RESULT = res

    out = np.zeros((T, H), np.float32)
    for e in range(E):
        n = counts[e]
        if n == 0:
            continue
        y = res.results[e]["out"].reshape(H, C).T[:n]  # [n, H]
        ids = token_ids[e]
        w_e = np.where(top_idx[ids] == e, top_w[ids], 0.0).sum(axis=1)
        out[ids] += w_e[:, None].astype(np.float32) * y
    return out.reshape(B, S, H), lb_loss
